# revision 31
# baseline (speedup 1.0000x reference)
"""MoE layer (top-2 of 8 experts, exact GELU) on 8 Trainium2 NeuronCores.

Strategy: expert parallelism. The router (0.006% of the FLOPs) runs on the
host; each core holds one expert's weights resident in SBUF and runs the
two big matmuls for the tokens routed to that expert:

    H^T = W1[e]^T @ X_e^T          (PE, bf16, accumulated over C in fp32)
    G   = GELU(H^T + b1)           (ACT, fused bias, bf16 out)
    Y   = G^T @ W2[e]              (PE, bf16, accumulated over D in fp32)

The host dispatches each expert's token batch pre-transposed ([C, cap]) in
bf16, then combines Y across the two selected experts per token with the
renormalized top-2 weights (plus the b2 term) in fp32.

Computing H transposed (d on partitions) makes the first matmul's output
directly usable as the second matmul's stationary operand -- no on-device
transposes anywhere.

DMA schedule (the whole game -- the PE stream itself runs at 99% of the
issue-rate roofline): the two HWDGE queues (sync, scalar) carry the
block-0 activations and w1 in 512 KB chunks interleaved in consumption
order, so the first matmul can start as soon as the first chunk lands
(~12 us) and w1 delivery (~350 GB/s across both queues) stays ahead of
the PE's 195 GB/s consumption. w2 rides the same queues strictly behind
w1 (in-queue ordering -- no dep edges needed) plus the late-starting
gpsimd SWDGE ring, all landing well before the second matmul of block 0
needs them. Nothing early-needed ever shares the bus with late-needed
bulk, which is what starved the PE for ~46 us in the naive schedule.
"""

import numpy as np
import ml_dtypes

B, T, C, D, E = 2, 2048, 1024, 4096, 8
N = B * T
TOP_K = 2
NT_BLOCK = 384          # token block width (matmul1 free dim)
TOK_TILE = 128          # token tile (matmul2 stationary free dim / psum partitions)
WC = 256                # w1 d-columns per DMA chunk (2 d-tiles, 512 KB)

_cache = {}


MAX_WAITS = 1  # this walrus build rejects >1 sync wait per instruction


def _install_tile_patch():
    """This container's walrus build rejects >MAX_WAITS sync waits on one
    instruction ("Too many sync wait commands"). Two fixes:
    1. The stock TileContext tail puts every outstanding proc-clock wait on
       a single Drain -- split across sync-engine NOPs, one wait each.
    2. Body instructions can come out of sem assignment with 3+ waits --
       peel the excess onto same-engine NOPs inserted just before."""
    import bass_rust
    import concourse.mybir as mybir
    from concourse import tile
    from concourse.vector_clock import ScopedClock

    if getattr(tile.TileContext, "_drain_patched", False):
        return

    def _patched(self, tick_clock, wait_clock):
        nc = self.nc
        ticks = list(tick_clock.global_clock)
        for p, t in enumerate(ticks):
            if t <= 0:
                continue
            vc = bass_rust.VectorClock()
            vc.require_at_least(p, t)
            nop = nc.sync.nop(nofuse=True, hint=f"tail_wait_p{p}")
            wait_clock.add_sem_waits(nop.ins, ScopedClock({None: vc}))
        nc.sync.drain()
        nc.all_engine_barrier()
        assert self.sems is not None
        popped = nc._tile_sem_poison_stack.pop()
        assert popped is self._sem_poison
        # Skip clear_and_free_semaphores + the second barrier: the
        # compiler's own epilogue zeroes the entire 256-sem file right
        # after this anyway, and this program is built once per cap --
        # the ~0.8 us of churn lands inside the measured exec window.

    tile.TileContext._drain_and_barrier = _patched

    orig_lower = tile.TileContext._lower_ordered_insts

    def _split_excess_waits(self, ordered):
        nc = self.nc
        for bb_name, insts in ordered.items():
            out = []
            for inst in insts:
                si = getattr(inst, "sync_info", None)
                if (
                    si is not None
                    and si.on_wait
                    and len(si.on_wait) > MAX_WAITS
                    and isinstance(inst, mybir.Instruction)
                    and inst.engine != mybir.EngineType.Unassigned
                ):
                    waits = list(si.on_wait)
                    excess, keep = waits[:-MAX_WAITS], waits[-MAX_WAITS:]
                    while excess:
                        chunk, excess = excess[:MAX_WAITS], excess[MAX_WAITS:]
                        nop = mybir.InstNoOp(
                            name=nc.get_next_instruction_name(),
                            sync_info=mybir.SyncInfo(on_wait=chunk, on_update=[]),
                            bass_nofuse=True,
                            engine=inst.engine,
                        )
                        nc.register_instruction(nop)
                        out.append(nop)
                    inst.sync_info = mybir.SyncInfo(
                        on_wait=keep, on_update=list(si.on_update or [])
                    )
                out.append(inst)
            insts[:] = out
        return orig_lower(self, ordered)

    tile.TileContext._lower_ordered_insts = _split_excess_waits
    tile.TileContext._drain_patched = True


def _blocks_of(cap):
    """Token-block widths covering cap (even). A smaller lead block (320)
    shrinks the first-matmul DMA gate (block-0 activations + first w1
    columns); the rest splits into equal-ish blocks <= NT_BLOCK. All
    blocks stay >= ~310 so LDWEIGHTS (~97 ns) hides under the matmul
    stream (bw/2.4 ns)."""
    if cap <= 576:
        return [cap]
    lead = 320
    rem = cap - lead
    n = -(-rem // NT_BLOCK)
    base = rem // n
    blocks = [lead]
    for i in range(n):
        b = base + (1 if i < rem - base * n else 0)
        blocks.append(b + (b & 1))  # keep even; fix drift on the last
    blocks[-1] = cap - sum(blocks[:-1])
    return blocks


def _build(cap):
    """Build the per-core Bass program for token capacity `cap` (even)."""
    import concourse.bass as bass
    import concourse.mybir as mybir
    import concourse.tile as tile
    from contextlib import ExitStack

    _install_tile_patch()

    bf16 = mybir.dt.bfloat16
    f32 = mybir.dt.float32
    KC = C // 128            # 8 contraction chunks for matmul1
    DT = D // 128            # 32 d-tiles / contraction chunks for matmul2
    NW = D // WC             # 16 w1 DMA chunks
    NQ = DT // 4             # 8 w2 DMA chunks
    blocks = _blocks_of(cap)

    # No partition-id: the program is core-agnostic (per-core data comes
    # in via the input buffers), and dropping it removes the per-engine
    # TENSOR_LOADs from the ~7 us fixed preamble.
    nc = bass.Bass(enable_partition_id=False)
    # Inputs are host-pre-tiled so every DMA is one partition-major
    # transfer with multi-KB contiguous runs per partition (small runs
    # tank DMA efficiency; tiny transfers serialize at ~600 ns on the
    # queue).
    #   xt : [128, KC, cap]   xt[p, kc, t]  = x^T[kc*128+p, t]
    #   w1 : [NW, 128, KC, WC] w1[c, p, kc, j] = w1[kc*128+p, c*WC+j]
    #   w2 : [NQ, 128, 4, C]  w2[q, p, a, c] = w2[(4q+a)*128+p, c]
    # xt is packed block-contiguous: block b at columns [KC*t0, KC*(t0+bw))
    # as (kc, t) so each per-block DMA is one contiguous run per partition.
    xt = nc.declare_dram_parameter("xt", [128, KC * cap], bf16, isOutput=False)
    w1h = nc.declare_dram_parameter(
        "w1h", [4, 128, KC, WC // 2], bf16, isOutput=False)
    w1 = nc.declare_dram_parameter(
        "w1", [NW - 2, 128, KC, WC], bf16, isOutput=False)
    w2 = nc.declare_dram_parameter("w2", [NQ, 128, 4, C], bf16, isOutput=False)
    b1t = nc.declare_dram_parameter("b1t", [128, DT], f32, isOutput=False)
    # output is y^T, cb-major: y[cb, p, t] = y^T[cb*128+p, t]
    y = nc.declare_dram_parameter("y", [C // 128, 128, cap], bf16, isOutput=True)

    with tile.TileContext(nc) as tc, ExitStack() as ctx:
        const = ctx.enter_context(tc.tile_pool(name="const", bufs=1))
        # Every DMA destination below is a CONTIGUOUS multi-KB run per
        # partition -- the DGE emits one packet per destination run, and
        # sub-KB packets cap a queue at ~115 GB/s (measured) vs ~150+ for
        # multi-KB ones. Hence w1 lives chunk-major in SBUF and each token
        # block gets its own xt tile. The first w1 chunk is further split
        # into two single-d-tile pieces (w1a) that land on different
        # queues, shrinking the first-matmul gate.
        w1a = const.tile([128, 4, KC, WC // 2], bf16)  # dt0-3 (2 KB runs)
        w1s = const.tile([128, NW - 2, KC, WC], bf16)  # dt4.. (4 KB runs)
        w2s = const.tile([128, DT, C], bf16)           # 8 KB runs per quad
        xbs = [const.tile([128, KC, bw], bf16, name=f"xb{i}")
               for i, bw in enumerate(blocks)]         # >=2.5 KB runs
        b1s = const.tile([128, DT], f32)

        def xt_dma(eng, blk, kc0, kc1):
            t0 = sum(blocks[:blk])
            bw = blocks[blk]
            return eng.dma_start(
                xbs[blk][:, kc0:kc1, :],
                xt[:, KC * t0 + kc0 * bw : KC * t0 + kc1 * bw]
                .rearrange("p (kc t) -> p kc t", kc=kc1 - kc0),
            )

        def w1a_dma(eng, s):
            return eng.dma_start(w1a[:, s, :, :], w1h[s])

        def w1_dma(eng, c):
            return eng.dma_start(w1s[:, c - 2, :, :], w1[c - 2])

        def w2_dma(eng, q):
            return eng.dma_start(w2s[:, 4 * q : 4 * (q + 1), :], w2[q])

        # DMA emission order == consumption order. Queue model (measured
        # via semaphore-update timelines):
        #  - each of the three queues (sync/scalar HWDGE, gpsimd SWDGE)
        #    is strict FIFO: transfers complete sequentially at ~90-110
        #    GB/s per queue, with a ~3-4.5 us first-transfer init (HWDGE
        #    data flows from ~8.5 us, SWDGE from ~10.5 us);
        #  - so w1 chunks are striped across the queues in consumption
        #    order, each queue's load sized against its FIFO position;
        #  - explicit dep edges between transfers starve the queues
        #    (relay gaps, no descriptor-gen overlap) -- use none;
        #  - each queue allows ~4 outstanding trigger instructions before
        #    blocking the engine; scalar (which runs the GELUs from
        #    ~18 us) gets only triggers whose slots free up before then.
        # w1 assignment solved against per-queue FIFO capacity (done_j ~
        # data_start + cum_bytes_j / rate; HWDGE data from ~8.4 us, SWDGE
        # ~10.5; first transfer rides the ~3 us init window "free"; the
        # second transfer on a queue runs at roughly half rate). dt0-dt3
        # travel as single-d-tile pieces so the earliest-needed weights
        # are never positioned deep in a FIFO. Block 0's d-tile loop is
        # PERMUTED to match this projected arrival order (the dt loop is
        # order-free), so ~1.5 us of jitter per chunk is absorbed without
        # a stall. scalar gets exactly 7 triggers -- slots 5-7 free up
        # before the GELU stream needs the engine.
        NB = C // 128          # 8 output-channel tiles for matmul2
        gpool = ctx.enter_context(tc.tile_pool(name="g", bufs=2))
        ps1 = ctx.enter_context(tc.tile_pool(name="ps1", bufs=2, space="PSUM"))
        ps2 = ctx.enter_context(tc.tile_pool(name="ps2", bufs=4, space="PSUM"))
        yev = ctx.enter_context(tc.tile_pool(name="yev", bufs=2))
        warm = ctx.enter_context(tc.tile_pool(name="warm", bufs=1))
        warmps = ctx.enter_context(
            tc.tile_pool(name="warmps", bufs=1, space="PSUM"))
        wsrc = warm.tile([128, NT_BLOCK], bf16)
        wact = warm.tile([128, 32], bf16)
        wps = warmps.tile([128, NT_BLOCK], f32)
        wps2 = warmps.tile([128, 32], f32)

        w1a_dma(nc.scalar, 0)
        xt_dma(nc.sync, 0, 0, KC // 2)
        nc.scalar.dma_start(b1s[:], b1t[:])
        w1a_dma(nc.gpsimd, 1)
        xt_dma(nc.sync, 0, KC // 2, KC)
        w1a_dma(nc.scalar, 2)
        w1a_dma(nc.sync, 3)
        w1_dma(nc.gpsimd, 2)
        w1_dma(nc.gpsimd, 3)
        w1_dma(nc.scalar, 4)

        # PE warm-up: the HAM clock gate needs ~3.4us of sustained matmul
        # activity to lift the PE from 1.2 to 2.4 GHz. Burn the initial
        # DMA wait on dummy matmuls over zeroed scratch so the real ones
        # start at full clock. Also preload the GELU table here (the
        # first ACTIVATE otherwise pays a ~1.3 us ACT_TABLE_LOAD in the
        # matmul1 critical path); it gets its own scratch psum (a WAR on
        # wps would stall the warmups) and sits in the scalar stream
        # before the slot-blocked DMA triggers (which execute at ~11-16us).
        nc.vector.memset(wsrc[:], 0.0)
        nc.tensor.matmul(wps2[:], wsrc[:, :128], wsrc[:, :32],
                         start=True, stop=True)
        nc.scalar.activation(wact[:], wps2[:],
                             mybir.ActivationFunctionType.Gelu)
        for _ in range(23):
            nc.tensor.matmul(wps[:], wsrc[:, :128], wsrc[:], start=True, stop=True)

        for c in (7, 10, 13):
            w1_dma(nc.scalar, c)
        for c in (5, 8, 11, 14, 15):
            w1_dma(nc.sync, c)
        for c in (6, 9, 12):
            w1_dma(nc.gpsimd, c)
        # block-pairing (below) defers matmul2-of-block-0 to T0+~75 us,
        # so w2 has a huge window: whole quads, behind w1 in-queue.
        xt_dma(nc.gpsimd, 1, 0, KC)
        for q, eng in ((0, nc.sync), (1, nc.gpsimd), (2, nc.sync),
                       (3, nc.gpsimd), (4, nc.sync), (5, nc.gpsimd),
                       (6, nc.sync), (7, nc.gpsimd)):
            w2_dma(eng, q)
        if len(blocks) > 2:
            xt_dma(nc.gpsimd, 2, 0, KC)

        # block-0 d-tiles in projected chunk-arrival order (see the DMA
        # schedule above); later blocks run after w1 fully lands.
        ORDER0 = [0, 1, 2, 3, 4, 5, 8, 9, 10, 11, 6, 7, 14, 15, 16, 17,
                  12, 13, 20, 21, 22, 23, 18, 19, 26, 27, 28, 29, 24, 25,
                  30, 31]

        def mm1_block(blk):
            bw = blocks[blk]
            g = gpool.tile([128, DT, bw], bf16, tag="g")
            for dt in (ORDER0 if blk == 0 else range(DT)):
                ph = ps1.tile([128, bw], f32, tag="ph")
                if dt < 4:
                    w1ap = lambda kc, dt=dt: w1a[:, dt, kc, :]
                else:
                    w1ap = lambda kc, dt=dt: w1s[
                        :, (dt - 4) // 2, kc,
                        128 * (dt % 2) : 128 * (dt % 2) + 128]
                for kc in range(KC):
                    nc.tensor.matmul(
                        ph[:],
                        w1ap(kc),
                        xbs[blk][:, kc, :],
                        start=(kc == 0),
                        stop=(kc == KC - 1),
                    )
                nc.scalar.activation(
                    g[:, dt, :], ph[:],
                    mybir.ActivationFunctionType.Gelu,
                    bias=b1s[:, dt : dt + 1],
                )
            return g

        def mm2_block(blk, g):
            bw = blocks[blk]
            t0 = sum(blocks[:blk])
            last = blk == len(blocks) - 1
            yt = yev.tile([128, NB, bw], bf16, tag="yt")
            for cb in range(NB):
                py = ps2.tile([128, bw], f32, tag="py",
                              name=f"py_b{blk}_c{cb}")
                for dt in range(DT):
                    nc.tensor.matmul(
                        py[:],
                        w2s[:, dt, 128 * cb : 128 * (cb + 1)],
                        g[:, dt, :],
                        start=(dt == 0),
                        stop=(dt == DT - 1),
                    )
                if last and cb == NB - 1:
                    # split the very last eviction+writeback so the final
                    # DMA after the last matmul is ~50 KB
                    for h in range(2):
                        sl = slice(h * (bw // 2),
                                   bw if h else bw // 2)
                        nc.vector.tensor_copy(yt[:, cb, sl], py[:, sl])
                        nc.sync.dma_start(y[cb, :, t0 + sl.start : t0 + sl.stop],
                                          yt[:, cb, sl])
                else:
                    nc.vector.tensor_copy(yt[:, cb, :], py[:])
                    if last:
                        # per-cb writeback so the final DMA after the last
                        # matmul is tiny (~100 KB) instead of a whole block
                        nc.sync.dma_start(y[cb, :, t0 : t0 + bw], yt[:, cb, :])
            if not last:
                nc.sync.dma_start(
                    y[:, :, t0 : t0 + bw].rearrange("cb p t -> p cb t"), yt[:]
                )

        # Block-pairing: run matmul1 of blocks 0+1 back to back before
        # any matmul2. This defers the first w2 consumption by a whole
        # mm1 phase (~40 us), turning the w2 8 MB burst (which cannot fit
        # in a single mm2 phase at 3-queue bandwidth) into a leisurely
        # background load. g holds two blocks -- same 2-buffer pool.
        if len(blocks) >= 2:
            g0 = mm1_block(0)
            g1 = mm1_block(1)
            mm2_block(0, g0)
            mm2_block(1, g1)
            rest = range(2, len(blocks))
        else:
            rest = range(len(blocks))
        for blk in rest:
            mm2_block(blk, mm1_block(blk))
    return nc


def _route(xf, w_router):
    """Host router: softmax over experts, top-2 (jax tie semantics:
    stable, lower index first), renormalize."""
    logits = xf @ w_router.T                       # [N, E] fp32
    m = logits.max(axis=-1, keepdims=True)
    p = np.exp(logits - m)
    p /= p.sum(axis=-1, keepdims=True)
    topi = np.argsort(-p, axis=-1, kind="stable")[:, :TOP_K]   # [N, 2]
    topw = np.take_along_axis(p, topi, axis=-1)
    topw = topw / topw.sum(axis=-1, keepdims=True)
    return topi.astype(np.int32), topw.astype(np.float32)


def _run_spmd(nc, in_maps, trace=False, trace_cores=None, tmpdir=None):
    from concourse.bass_utils import run_bass_kernel_spmd

    return run_bass_kernel_spmd(
        nc, in_maps, core_ids=list(range(E)),
        trace=trace, trace_cores=trace_cores, tmpdir=tmpdir,
    )


# test.py hooks: set TRACE=True (and optionally TRACE_CORES/TRACE_DIR)
# before calling kernel() to capture an NTFF profile of the run.
TRACE = False
TRACE_CORES = None
TRACE_DIR = None
LAST_RESULT = None


def kernel(x, w_router, w1, b1, w2, b2):
    global LAST_RESULT
    x = np.asarray(x, dtype=np.float32)
    w_router = np.asarray(w_router, dtype=np.float32)
    w1 = np.asarray(w1, dtype=np.float32)
    b1 = np.asarray(b1, dtype=np.float32)
    w2 = np.asarray(w2, dtype=np.float32)
    b2 = np.asarray(b2, dtype=np.float32)

    xf = x.reshape(N, C)
    topi, topw = _route(xf, w_router)

    # token rows routed to each expert (each token appears in exactly 2)
    sel = [np.nonzero((topi == e).any(axis=-1))[0] for e in range(E)]
    max_cnt = max(len(s) for s in sel)
    cap = max(128, -(-max_cnt // 2) * 2)

    if cap not in _cache:
        _cache[cap] = _build(cap)
    nc = _cache[cap]

    bf16 = ml_dtypes.bfloat16
    xf_bf = xf.astype(bf16)
    in_maps = []
    for e in range(E):
        rows = sel[e]
        xt = np.zeros((C, cap), dtype=bf16)
        xt[:, : len(rows)] = xf_bf[rows].T
        # layouts documented in _build; xt packed block-contiguous
        xk = xt.reshape(C // 128, 128, cap)
        parts, t0 = [], 0
        for bw in _blocks_of(cap):
            parts.append(
                xk[:, :, t0 : t0 + bw].transpose(1, 0, 2).reshape(128, -1))
            t0 += bw
        xtt = np.ascontiguousarray(np.concatenate(parts, axis=1))
        w1t = w1[e].astype(bf16).reshape(C // 128, 128, D // WC, WC)
        w1t = w1t.transpose(2, 1, 0, 3)                  # [NW, 128, KC, WC]
        # first two chunks (dt0-3) travel as 4 single-d-tile pieces
        w1h = w1t[:2].reshape(2, 128, C // 128, 2, WC // 2)
        w1h = np.ascontiguousarray(
            w1h.transpose(0, 3, 1, 2, 4).reshape(4, 128, C // 128, WC // 2))
        w2t = w2[e].astype(bf16).reshape(D // 512, 4, 128, C)
        w2t = np.ascontiguousarray(w2t.transpose(0, 2, 1, 3))
        in_maps.append({
            "xt": xtt,
            "w1h": w1h,
            "w1": np.ascontiguousarray(w1t[2:]),
            "w2": w2t,
            "b1t": np.ascontiguousarray(b1[e].reshape(D // 128, 128).T),
        })

    res = _run_spmd(nc, in_maps, trace=TRACE, trace_cores=TRACE_CORES,
                    tmpdir=TRACE_DIR)
    LAST_RESULT = res

    out = np.zeros((N, C), dtype=np.float32)
    for e in range(E):
        rows = sel[e]
        if len(rows) == 0:
            continue
        ye = np.asarray(res.results[e]["y"], dtype=np.float32)
        ye = ye.reshape(C, -1).T[: len(rows)]          # y^T, cb-major -> [n, C]
        # weight of expert e for each selected token
        is_e = topi[rows] == e               # [n_e, 2]
        wgt = (topw[rows] * is_e).sum(axis=-1)
        out[rows] += wgt[:, None] * ye
    # b2 enters after the expert matmul, inside the weighted combine
    out += (topw[:, :, None] * b2[topi]).sum(axis=1)
    return out.reshape(B, T, C)


# revision 32
# speedup vs baseline: 1.0171x; 1.0171x over previous
"""MoE layer (top-2 of 8 experts, exact GELU) on 8 Trainium2 NeuronCores.

Strategy: expert parallelism. The router (0.006% of the FLOPs) runs on the
host; each core holds one expert's weights resident in SBUF and runs the
two big matmuls for the tokens routed to that expert:

    H^T = W1[e]^T @ X_e^T          (PE, bf16, accumulated over C in fp32)
    G   = GELU(H^T + b1)           (ACT, fused bias, bf16 out)
    Y   = G^T @ W2[e]              (PE, bf16, accumulated over D in fp32)

The host dispatches each expert's token batch pre-transposed ([C, cap]) in
bf16, then combines Y across the two selected experts per token with the
renormalized top-2 weights (plus the b2 term) in fp32.

Computing H transposed (d on partitions) makes the first matmul's output
directly usable as the second matmul's stationary operand -- no on-device
transposes anywhere.

DMA schedule (the whole game -- the PE stream itself runs at 99% of the
issue-rate roofline): the two HWDGE queues (sync, scalar) carry the
block-0 activations and w1 in 512 KB chunks interleaved in consumption
order, so the first matmul can start as soon as the first chunk lands
(~12 us) and w1 delivery (~350 GB/s across both queues) stays ahead of
the PE's 195 GB/s consumption. w2 rides the same queues strictly behind
w1 (in-queue ordering -- no dep edges needed) plus the late-starting
gpsimd SWDGE ring, all landing well before the second matmul of block 0
needs them. Nothing early-needed ever shares the bus with late-needed
bulk, which is what starved the PE for ~46 us in the naive schedule.
"""

import numpy as np
import ml_dtypes

B, T, C, D, E = 2, 2048, 1024, 4096, 8
N = B * T
TOP_K = 2
NT_BLOCK = 384          # token block width (matmul1 free dim)
TOK_TILE = 128          # token tile (matmul2 stationary free dim / psum partitions)
WC = 256                # w1 d-columns per DMA chunk (2 d-tiles, 512 KB)

_cache = {}


MAX_WAITS = 1  # this walrus build rejects >1 sync wait per instruction


def _install_tile_patch():
    """This container's walrus build rejects >MAX_WAITS sync waits on one
    instruction ("Too many sync wait commands"). Two fixes:
    1. The stock TileContext tail puts every outstanding proc-clock wait on
       a single Drain -- split across sync-engine NOPs, one wait each.
    2. Body instructions can come out of sem assignment with 3+ waits --
       peel the excess onto same-engine NOPs inserted just before."""
    import bass_rust
    import concourse.mybir as mybir
    from concourse import tile
    from concourse.vector_clock import ScopedClock

    if getattr(tile.TileContext, "_drain_patched", False):
        return

    def _patched(self, tick_clock, wait_clock):
        nc = self.nc
        ticks = list(tick_clock.global_clock)
        for p, t in enumerate(ticks):
            if t <= 0:
                continue
            vc = bass_rust.VectorClock()
            vc.require_at_least(p, t)
            nop = nc.sync.nop(nofuse=True, hint=f"tail_wait_p{p}")
            wait_clock.add_sem_waits(nop.ins, ScopedClock({None: vc}))
        nc.sync.drain()
        nc.all_engine_barrier()
        assert self.sems is not None
        popped = nc._tile_sem_poison_stack.pop()
        assert popped is self._sem_poison
        # Skip clear_and_free_semaphores + the second barrier: the
        # compiler's own epilogue zeroes the entire 256-sem file right
        # after this anyway, and this program is built once per cap --
        # the ~0.8 us of churn lands inside the measured exec window.

    tile.TileContext._drain_and_barrier = _patched

    orig_lower = tile.TileContext._lower_ordered_insts

    def _split_excess_waits(self, ordered):
        nc = self.nc
        for bb_name, insts in ordered.items():
            out = []
            for inst in insts:
                si = getattr(inst, "sync_info", None)
                if (
                    si is not None
                    and si.on_wait
                    and len(si.on_wait) > MAX_WAITS
                    and isinstance(inst, mybir.Instruction)
                    and inst.engine != mybir.EngineType.Unassigned
                ):
                    waits = list(si.on_wait)
                    excess, keep = waits[:-MAX_WAITS], waits[-MAX_WAITS:]
                    while excess:
                        chunk, excess = excess[:MAX_WAITS], excess[MAX_WAITS:]
                        nop = mybir.InstNoOp(
                            name=nc.get_next_instruction_name(),
                            sync_info=mybir.SyncInfo(on_wait=chunk, on_update=[]),
                            bass_nofuse=True,
                            engine=inst.engine,
                        )
                        nc.register_instruction(nop)
                        out.append(nop)
                    inst.sync_info = mybir.SyncInfo(
                        on_wait=keep, on_update=list(si.on_update or [])
                    )
                out.append(inst)
            insts[:] = out
        return orig_lower(self, ordered)

    tile.TileContext._lower_ordered_insts = _split_excess_waits
    tile.TileContext._drain_patched = True


def _blocks_of(cap):
    """Token-block widths covering cap (even). A smaller lead block (320)
    shrinks the first-matmul DMA gate (block-0 activations + first w1
    columns); the rest splits into equal-ish blocks <= NT_BLOCK. All
    blocks stay >= ~310 so LDWEIGHTS (~97 ns) hides under the matmul
    stream (bw/2.4 ns)."""
    if cap <= 576:
        return [cap]
    lead = 320
    rem = cap - lead
    n = -(-rem // NT_BLOCK)
    base = rem // n
    blocks = [lead]
    for i in range(n):
        b = base + (1 if i < rem - base * n else 0)
        blocks.append(b + (b & 1))  # keep even; fix drift on the last
    blocks[-1] = cap - sum(blocks[:-1])
    return blocks


def _build(cap):
    """Build the per-core Bass program for token capacity `cap` (even)."""
    import concourse.bass as bass
    import concourse.mybir as mybir
    import concourse.tile as tile
    from contextlib import ExitStack

    _install_tile_patch()

    bf16 = mybir.dt.bfloat16
    f32 = mybir.dt.float32
    KC = C // 128            # 8 contraction chunks for matmul1
    DT = D // 128            # 32 d-tiles / contraction chunks for matmul2
    NW = D // WC             # 16 w1 DMA chunks
    NQ = DT // 4             # 8 w2 DMA chunks
    blocks = _blocks_of(cap)

    # No partition-id: the program is core-agnostic (per-core data comes
    # in via the input buffers), and dropping it removes the per-engine
    # TENSOR_LOADs from the ~7 us fixed preamble.
    nc = bass.Bass(enable_partition_id=False)
    # Inputs are host-pre-tiled so every DMA is one partition-major
    # transfer with multi-KB contiguous runs per partition (small runs
    # tank DMA efficiency; tiny transfers serialize at ~600 ns on the
    # queue).
    #   xt : [128, KC, cap]   xt[p, kc, t]  = x^T[kc*128+p, t]
    #   w1 : [NW, 128, KC, WC] w1[c, p, kc, j] = w1[kc*128+p, c*WC+j]
    #   w2 : [NQ, 128, 4, C]  w2[q, p, a, c] = w2[(4q+a)*128+p, c]
    # xt is packed block-contiguous: block b at columns [KC*t0, KC*(t0+bw))
    # as (kc, t) so each per-block DMA is one contiguous run per partition.
    xt = nc.declare_dram_parameter("xt", [128, KC * cap], bf16, isOutput=False)
    w1h = nc.declare_dram_parameter(
        "w1h", [4, 128, KC, WC // 2], bf16, isOutput=False)
    w1 = nc.declare_dram_parameter(
        "w1", [NW - 2, 128, KC, WC], bf16, isOutput=False)
    w2 = nc.declare_dram_parameter("w2", [NQ, 128, 4, C], bf16, isOutput=False)
    b1t = nc.declare_dram_parameter("b1t", [128, DT], f32, isOutput=False)
    # output is y^T, cb-major: y[cb, p, t] = y^T[cb*128+p, t]
    y = nc.declare_dram_parameter("y", [C // 128, 128, cap], bf16, isOutput=True)

    with tile.TileContext(nc) as tc, ExitStack() as ctx:
        const = ctx.enter_context(tc.tile_pool(name="const", bufs=1))
        # Every DMA destination below is a CONTIGUOUS multi-KB run per
        # partition -- the DGE emits one packet per destination run, and
        # sub-KB packets cap a queue at ~115 GB/s (measured) vs ~150+ for
        # multi-KB ones. Hence w1 lives chunk-major in SBUF and each token
        # block gets its own xt tile. The first w1 chunk is further split
        # into two single-d-tile pieces (w1a) that land on different
        # queues, shrinking the first-matmul gate.
        w1a = const.tile([128, 4, KC, WC // 2], bf16)  # dt0-3 (2 KB runs)
        w1s = const.tile([128, NW - 2, KC, WC], bf16)  # dt4.. (4 KB runs)
        w2s = const.tile([128, DT, C], bf16)           # 8 KB runs per quad
        xbs = [const.tile([128, KC, bw], bf16, name=f"xb{i}")
               for i, bw in enumerate(blocks)]         # >=2.5 KB runs
        b1s = const.tile([128, DT], f32)

        def xt_dma(eng, blk, kc0, kc1):
            t0 = sum(blocks[:blk])
            bw = blocks[blk]
            return eng.dma_start(
                xbs[blk][:, kc0:kc1, :],
                xt[:, KC * t0 + kc0 * bw : KC * t0 + kc1 * bw]
                .rearrange("p (kc t) -> p kc t", kc=kc1 - kc0),
            )

        def w1a_dma(eng, s):
            return eng.dma_start(w1a[:, s, :, :], w1h[s])

        def w1_dma(eng, c):
            return eng.dma_start(w1s[:, c - 2, :, :], w1[c - 2])

        def w2_dma(eng, q):
            return eng.dma_start(w2s[:, 4 * q : 4 * (q + 1), :], w2[q])

        # DMA emission order == consumption order. Queue model (measured
        # via semaphore-update timelines):
        #  - each of the three queues (sync/scalar HWDGE, gpsimd SWDGE)
        #    is strict FIFO: transfers complete sequentially at ~90-110
        #    GB/s per queue, with a ~3-4.5 us first-transfer init (HWDGE
        #    data flows from ~8.5 us, SWDGE from ~10.5 us);
        #  - so w1 chunks are striped across the queues in consumption
        #    order, each queue's load sized against its FIFO position;
        #  - explicit dep edges between transfers starve the queues
        #    (relay gaps, no descriptor-gen overlap) -- use none;
        #  - each queue allows ~4 outstanding trigger instructions before
        #    blocking the engine; scalar (which runs the GELUs from
        #    ~18 us) gets only triggers whose slots free up before then.
        # w1 assignment solved against per-queue FIFO capacity (done_j ~
        # data_start + cum_bytes_j / rate; HWDGE data from ~8.4 us, SWDGE
        # ~10.5; first transfer rides the ~3 us init window "free"; the
        # second transfer on a queue runs at roughly half rate). dt0-dt3
        # travel as single-d-tile pieces so the earliest-needed weights
        # are never positioned deep in a FIFO. Block 0's d-tile loop is
        # PERMUTED to match this projected arrival order (the dt loop is
        # order-free), so ~1.5 us of jitter per chunk is absorbed without
        # a stall. scalar gets exactly 7 triggers -- slots 5-7 free up
        # before the GELU stream needs the engine.
        NB = C // 128          # 8 output-channel tiles for matmul2
        gpool = ctx.enter_context(tc.tile_pool(name="g", bufs=2))
        ps1 = ctx.enter_context(tc.tile_pool(name="ps1", bufs=2, space="PSUM"))
        ps2 = ctx.enter_context(tc.tile_pool(name="ps2", bufs=4, space="PSUM"))
        yev = ctx.enter_context(tc.tile_pool(name="yev", bufs=2))
        warm = ctx.enter_context(tc.tile_pool(name="warm", bufs=1))
        warmps = ctx.enter_context(
            tc.tile_pool(name="warmps", bufs=1, space="PSUM"))
        wsrc = warm.tile([128, NT_BLOCK], bf16)
        wact = warm.tile([128, 32], bf16)
        wps = warmps.tile([128, NT_BLOCK], f32)
        wps2 = warmps.tile([128, 32], f32)

        w1a_dma(nc.scalar, 0)
        xt_dma(nc.sync, 0, 0, KC // 2)
        nc.scalar.dma_start(b1s[:], b1t[:])
        w1a_dma(nc.gpsimd, 1)
        xt_dma(nc.sync, 0, KC // 2, KC)
        w1a_dma(nc.scalar, 2)
        w1a_dma(nc.sync, 3)
        w1_dma(nc.gpsimd, 2)
        w1_dma(nc.gpsimd, 3)
        w1_dma(nc.scalar, 4)

        # PE warm-up: the HAM clock gate needs ~3.4us of sustained matmul
        # activity to lift the PE from 1.2 to 2.4 GHz. Burn the initial
        # DMA wait on dummy matmuls over zeroed scratch so the real ones
        # start at full clock. Also preload the GELU table here (the
        # first ACTIVATE otherwise pays a ~1.3 us ACT_TABLE_LOAD in the
        # matmul1 critical path); it gets its own scratch psum (a WAR on
        # wps would stall the warmups) and sits in the scalar stream
        # before the slot-blocked DMA triggers (which execute at ~11-16us).
        nc.vector.memset(wsrc[:], 0.0)
        nc.tensor.matmul(wps2[:], wsrc[:, :128], wsrc[:, :32],
                         start=True, stop=True)
        nc.scalar.activation(wact[:], wps2[:],
                             mybir.ActivationFunctionType.Gelu)
        for _ in range(25):
            nc.tensor.matmul(wps[:], wsrc[:, :128], wsrc[:], start=True, stop=True)

        for c in (7, 10, 13):
            w1_dma(nc.scalar, c)
        for c in (5, 8, 11, 14, 15):
            w1_dma(nc.sync, c)
        for c in (6, 9, 12):
            w1_dma(nc.gpsimd, c)
        # block-pairing (below) defers matmul2-of-block-0 to T0+~75 us,
        # so w2 has a huge window: whole quads, behind w1 in-queue.
        xt_dma(nc.gpsimd, 1, 0, KC)
        for q, eng in ((0, nc.sync), (1, nc.gpsimd), (2, nc.sync),
                       (3, nc.gpsimd), (4, nc.sync), (5, nc.gpsimd),
                       (6, nc.sync), (7, nc.gpsimd)):
            w2_dma(eng, q)
        if len(blocks) > 2:
            xt_dma(nc.gpsimd, 2, 0, KC)

        # block-0 d-tiles in projected chunk-arrival order (see the DMA
        # schedule above); later blocks run after w1 fully lands.
        ORDER0 = [0, 1, 2, 3, 4, 5, 8, 9, 10, 11, 6, 7, 14, 15, 16, 17,
                  12, 13, 20, 21, 22, 23, 18, 19, 26, 27, 28, 29, 24, 25,
                  30, 31]

        def mm1_block(blk):
            bw = blocks[blk]
            g = gpool.tile([128, DT, bw], bf16, tag="g")
            for dt in (ORDER0 if blk == 0 else range(DT)):
                ph = ps1.tile([128, bw], f32, tag="ph")
                if dt < 4:
                    w1ap = lambda kc, dt=dt: w1a[:, dt, kc, :]
                else:
                    w1ap = lambda kc, dt=dt: w1s[
                        :, (dt - 4) // 2, kc,
                        128 * (dt % 2) : 128 * (dt % 2) + 128]
                for kc in range(KC):
                    nc.tensor.matmul(
                        ph[:],
                        w1ap(kc),
                        xbs[blk][:, kc, :],
                        start=(kc == 0),
                        stop=(kc == KC - 1),
                    )
                nc.scalar.activation(
                    g[:, dt, :], ph[:],
                    mybir.ActivationFunctionType.Gelu,
                    bias=b1s[:, dt : dt + 1],
                )
            return g

        def mm2_block(blk, g):
            bw = blocks[blk]
            t0 = sum(blocks[:blk])
            last = blk == len(blocks) - 1
            yt = yev.tile([128, NB, bw], bf16, tag="yt")
            for cb in range(NB):
                py = ps2.tile([128, bw], f32, tag="py",
                              name=f"py_b{blk}_c{cb}")
                for dt in range(DT):
                    nc.tensor.matmul(
                        py[:],
                        w2s[:, dt, 128 * cb : 128 * (cb + 1)],
                        g[:, dt, :],
                        start=(dt == 0),
                        stop=(dt == DT - 1),
                    )
                if last and cb == NB - 1:
                    # split the very last eviction+writeback so the final
                    # DMA after the last matmul is ~50 KB
                    for h in range(2):
                        sl = slice(h * (bw // 2),
                                   bw if h else bw // 2)
                        nc.vector.tensor_copy(yt[:, cb, sl], py[:, sl])
                        nc.sync.dma_start(y[cb, :, t0 + sl.start : t0 + sl.stop],
                                          yt[:, cb, sl])
                else:
                    nc.vector.tensor_copy(yt[:, cb, :], py[:])
                    if last:
                        # per-cb writeback so the final DMA after the last
                        # matmul is tiny (~100 KB) instead of a whole block
                        nc.sync.dma_start(y[cb, :, t0 : t0 + bw], yt[:, cb, :])
            if not last:
                nc.sync.dma_start(
                    y[:, :, t0 : t0 + bw].rearrange("cb p t -> p cb t"), yt[:]
                )

        # Block-pairing: run matmul1 of blocks 0+1 back to back before
        # any matmul2. This defers the first w2 consumption by a whole
        # mm1 phase (~40 us), turning the w2 8 MB burst (which cannot fit
        # in a single mm2 phase at 3-queue bandwidth) into a leisurely
        # background load. g holds two blocks -- same 2-buffer pool.
        if len(blocks) >= 2:
            g0 = mm1_block(0)
            g1 = mm1_block(1)
            mm2_block(0, g0)
            mm2_block(1, g1)
            rest = range(2, len(blocks))
        else:
            rest = range(len(blocks))
        for blk in rest:
            mm2_block(blk, mm1_block(blk))
    return nc


def _route(xf, w_router):
    """Host router: softmax over experts, top-2 (jax tie semantics:
    stable, lower index first), renormalize."""
    logits = xf @ w_router.T                       # [N, E] fp32
    m = logits.max(axis=-1, keepdims=True)
    p = np.exp(logits - m)
    p /= p.sum(axis=-1, keepdims=True)
    topi = np.argsort(-p, axis=-1, kind="stable")[:, :TOP_K]   # [N, 2]
    topw = np.take_along_axis(p, topi, axis=-1)
    topw = topw / topw.sum(axis=-1, keepdims=True)
    return topi.astype(np.int32), topw.astype(np.float32)


def _run_spmd(nc, in_maps, trace=False, trace_cores=None, tmpdir=None):
    from concourse.bass_utils import run_bass_kernel_spmd

    return run_bass_kernel_spmd(
        nc, in_maps, core_ids=list(range(E)),
        trace=trace, trace_cores=trace_cores, tmpdir=tmpdir,
    )


# test.py hooks: set TRACE=True (and optionally TRACE_CORES/TRACE_DIR)
# before calling kernel() to capture an NTFF profile of the run.
TRACE = False
TRACE_CORES = None
TRACE_DIR = None
LAST_RESULT = None


def kernel(x, w_router, w1, b1, w2, b2):
    global LAST_RESULT
    x = np.asarray(x, dtype=np.float32)
    w_router = np.asarray(w_router, dtype=np.float32)
    w1 = np.asarray(w1, dtype=np.float32)
    b1 = np.asarray(b1, dtype=np.float32)
    w2 = np.asarray(w2, dtype=np.float32)
    b2 = np.asarray(b2, dtype=np.float32)

    xf = x.reshape(N, C)
    topi, topw = _route(xf, w_router)

    # token rows routed to each expert (each token appears in exactly 2)
    sel = [np.nonzero((topi == e).any(axis=-1))[0] for e in range(E)]
    max_cnt = max(len(s) for s in sel)
    cap = max(128, -(-max_cnt // 2) * 2)

    if cap not in _cache:
        _cache[cap] = _build(cap)
    nc = _cache[cap]

    bf16 = ml_dtypes.bfloat16
    xf_bf = xf.astype(bf16)
    in_maps = []
    for e in range(E):
        rows = sel[e]
        xt = np.zeros((C, cap), dtype=bf16)
        xt[:, : len(rows)] = xf_bf[rows].T
        # layouts documented in _build; xt packed block-contiguous
        xk = xt.reshape(C // 128, 128, cap)
        parts, t0 = [], 0
        for bw in _blocks_of(cap):
            parts.append(
                xk[:, :, t0 : t0 + bw].transpose(1, 0, 2).reshape(128, -1))
            t0 += bw
        xtt = np.ascontiguousarray(np.concatenate(parts, axis=1))
        w1t = w1[e].astype(bf16).reshape(C // 128, 128, D // WC, WC)
        w1t = w1t.transpose(2, 1, 0, 3)                  # [NW, 128, KC, WC]
        # first two chunks (dt0-3) travel as 4 single-d-tile pieces
        w1h = w1t[:2].reshape(2, 128, C // 128, 2, WC // 2)
        w1h = np.ascontiguousarray(
            w1h.transpose(0, 3, 1, 2, 4).reshape(4, 128, C // 128, WC // 2))
        w2t = w2[e].astype(bf16).reshape(D // 512, 4, 128, C)
        w2t = np.ascontiguousarray(w2t.transpose(0, 2, 1, 3))
        in_maps.append({
            "xt": xtt,
            "w1h": w1h,
            "w1": np.ascontiguousarray(w1t[2:]),
            "w2": w2t,
            "b1t": np.ascontiguousarray(b1[e].reshape(D // 128, 128).T),
        })

    res = _run_spmd(nc, in_maps, trace=TRACE, trace_cores=TRACE_CORES,
                    tmpdir=TRACE_DIR)
    LAST_RESULT = res

    out = np.zeros((N, C), dtype=np.float32)
    for e in range(E):
        rows = sel[e]
        if len(rows) == 0:
            continue
        ye = np.asarray(res.results[e]["y"], dtype=np.float32)
        ye = ye.reshape(C, -1).T[: len(rows)]          # y^T, cb-major -> [n, C]
        # weight of expert e for each selected token
        is_e = topi[rows] == e               # [n_e, 2]
        wgt = (topw[rows] * is_e).sum(axis=-1)
        out[rows] += wgt[:, None] * ye
    # b2 enters after the expert matmul, inside the weighted combine
    out += (topw[:, :, None] * b2[topi]).sum(axis=1)
    return out.reshape(B, T, C)
